# revision 9
# baseline (speedup 1.0000x reference)
"""JointEdgeSegLoss Trainium2 kernel (v6: PE-matmul class sums, fp16/fp8,
host-side f-major layout).

Strategy (data-parallel over batch+rows, 8 cores):
  - core k handles image n=k//2, row-half h=k%2 (294912 pixels), laid out
    [P=128 partitions, Q=2304 free], 6 chunks of F=384.
  - Host pre-packs x per core twice: xs = fp16 [P, Q, 20] f-major
    (slots 0..18 = class logits, slot 19 = 1.0) for the matmul stationary,
    and xc = fp8e4m3 [P, C, Q] c-major feeding exp (only used for lse).
  - Device: ACT exp -> EB (c-major); DVE contiguous tree-add -> S;
    ACT ln -> LSE; Pool copies LSE into stationary slot 20.
  - All per-(class,pixel) sums via the PE: per f-column
      stationary XF[:,f,:] = [x(19) | 1 | lse]    (6 f packed = 126 cols)
      moving    OH[:,:,f]  = [onehot_t | onehot_tv | bce | tm | bce*tm]
    accumulate [126, 246] in PSUM over all 2304 columns. Host extracts
      T1[c]=sum (t==c) x[c], L1[c]=sum (t==c) lse, B1[c]=count(t==c)
    (plus tv family and bce sums), then S1 = T1 - L1 etc.
  - One-hots on DVE at 2x fp16: TT is_equal vs IOTA const; oh_tv = oh_t*gt.
  - Host combines tiny per-core partials in float64 (the "all-reduce").

Self-contained: hardcodes all shapes; only imports the runtime (concourse).
"""

import numpy as np

import concourse.bass as bass
import concourse.bacc as bacc
import concourse.mybir as mybir
import concourse.tile as tile
from concourse import bass_utils

F32 = mybir.dt.float32
I32 = mybir.dt.int32
FP16 = mybir.dt.float16
FP8 = mybir.dt.float8e4
ALU = mybir.AluOpType
ACTF = mybir.ActivationFunctionType

C = 19
N, H, W = 4, 768, 768
HW = H * W
NCORES = 8
M = N * HW // NCORES            # 294912 pixels per core
P = 128
Q = M // P                      # 2304
F = 384                         # pixels-per-partition per chunk
NCH = Q // F                    # 6 chunks
PK = 6                          # f-columns packed per matmul
NST = C + 2                     # stationary slots: x[19] | ones | lse
NMV = 2 * C + 3                 # moving slots: oh_t | oh_tv | bce | tm | bce*tm
NRW = NST * PK                  # psum rows    126
NCL = NMV * PK                  # psum columns 246
EDGE_THRESH = 0.8


def build_program():
    nc = bacc.Bacc("TRN2", target_bir_lowering=False, debug=False)

    xs = nc.dram_tensor("xs", [P, Q, C + 1], FP16, kind="ExternalInput")
    xc = nc.dram_tensor("xc", [P, C, Q], FP8, kind="ExternalInput")
    ts = nc.dram_tensor("ts", [P, Q], I32, kind="ExternalInput")
    es = nc.dram_tensor("es", [P, Q], F32, kind="ExternalInput")
    ms = nc.dram_tensor("ms", [P, Q], I32, kind="ExternalInput")
    acc_d = nc.dram_tensor("acc", [NRW, NCL], F32, kind="ExternalOutput")

    with tile.TileContext(nc) as tc:
        with (
            tc.tile_pool(name="xp", bufs=2) as xp,
            tc.tile_pool(name="ebp", bufs=2) as ebp,
            tc.tile_pool(name="ohp", bufs=2) as ohp,
            tc.tile_pool(name="mp", bufs=2) as mp,
            tc.tile_pool(name="sp", bufs=2) as sp,
            tc.tile_pool(name="cst", bufs=1) as cst,
            tc.tile_pool(name="ps", bufs=1, space=bass.MemorySpace.PSUM) as psp,
        ):
            IOTA = cst.tile([P, C, F], FP16, tag="iota")
            for c in range(C):
                nc.gpsimd.memset(IOTA[:, c, :], float(c))

            acc = psp.tile([NRW, NCL], F32, tag="acc")

            for k in range(NCH):
                f0 = k * F

                XF = xp.tile([P, F, NST], FP16, tag="XF")
                nc.sync.dma_start(
                    XF[:, :, 0:C + 1], xs.ap()[:, f0:f0 + F, :])
                XC = xp.tile([P, C, F], FP8, tag="XC")
                nc.sync.dma_start(XC[:], xc.ap()[:, :, f0:f0 + F])
                T = mp.tile([P, F], I32, tag="T")
                nc.sync.dma_start(T[:], ts.ap()[:, f0:f0 + F])
                E = mp.tile([P, F], F32, tag="E")
                nc.sync.dma_start(E[:], es.ap()[:, f0:f0 + F])
                Mm = mp.tile([P, F], I32, tag="Mm")
                nc.sync.dma_start(Mm[:], ms.ap()[:, f0:f0 + F])

                # ---- log-softmax denominator (c-major, contiguous tree) ----
                EB = ebp.tile([P, C, F], FP16, tag="EB")
                nc.scalar.activation(EB[:], XC[:], ACTF.Exp)
                nc.vector.tensor_tensor(
                    out=EB[:, 0:9, :], in0=EB[:, 0:9, :], in1=EB[:, 9:18, :],
                    op=ALU.add)
                nc.vector.tensor_tensor(
                    out=EB[:, 0:4, :], in0=EB[:, 0:4, :], in1=EB[:, 4:8, :],
                    op=ALU.add)
                nc.vector.tensor_tensor(
                    out=EB[:, 0:2, :], in0=EB[:, 0:2, :], in1=EB[:, 2:4, :],
                    op=ALU.add)
                nc.vector.tensor_tensor(
                    out=EB[:, 0:1, :], in0=EB[:, 0:1, :], in1=EB[:, 1:2, :],
                    op=ALU.add)
                nc.vector.tensor_tensor(
                    out=EB[:, 0:1, :], in0=EB[:, 0:1, :], in1=EB[:, 8:9, :],
                    op=ALU.add)
                nc.vector.tensor_tensor(
                    out=EB[:, 0:1, :], in0=EB[:, 0:1, :], in1=EB[:, 18:19, :],
                    op=ALU.add)
                LSE = sp.tile([P, F], FP16, tag="LSE")
                nc.scalar.activation(LSE[:], EB[:, 0, :], ACTF.Ln)
                # lse -> stationary slot 20 (strided column, on Pool)
                nc.gpsimd.tensor_copy(
                    XF[:, :, C + 1:NST], LSE[:].unsqueeze(2))

                # ---- one-hots (c-major) ----
                Tf = sp.tile([P, F], FP16, tag="Tf")
                nc.vector.tensor_copy(Tf[:], T[:])
                gt = sp.tile([P, F], FP16, tag="gt")
                nc.vector.tensor_scalar(
                    gt[:], E[:], EDGE_THRESH, None, op0=ALU.is_gt)

                OH = ohp.tile([P, NMV, F], FP16, tag="OH")
                nc.vector.tensor_tensor(
                    out=OH[:, 0:C, :],
                    in0=Tf[:].unsqueeze(1).broadcast_to([P, C, F]),
                    in1=IOTA[:], op=ALU.is_equal)
                nc.vector.tensor_tensor(
                    out=OH[:, C:2 * C, :], in0=OH[:, 0:C, :],
                    in1=gt[:].unsqueeze(1).broadcast_to([P, C, F]),
                    op=ALU.mult)

                # ---- bce terms into OH slots 38..40 ----
                # bce = relu(E) - E*tm + softplus(-|E|)
                tm = sp.tile([P, F], FP16, tag="tm")
                nc.gpsimd.tensor_copy(tm[:], Mm[:])
                ab = sp.tile([P, F], FP16, tag="ab")
                nc.scalar.activation(ab[:], E[:], ACTF.Abs)
                en = sp.tile([P, F], FP16, tag="en")
                nc.scalar.activation(en[:], ab[:], ACTF.Exp, scale=-1.0)
                l1p = sp.tile([P, F], FP16, tag="l1p")
                nc.scalar.activation(l1p[:], en[:], ACTF.Ln, bias=1.0)
                r = sp.tile([P, F], FP16, tag="r")
                nc.vector.tensor_scalar(r[:], E[:], 0.0, None, op0=ALU.max)
                q = sp.tile([P, F], FP16, tag="q")
                nc.gpsimd.tensor_tensor(out=q[:], in0=E[:], in1=tm[:],
                                        op=ALU.mult)
                b1 = sp.tile([P, F], FP16, tag="b1")
                nc.gpsimd.tensor_tensor(out=b1[:], in0=r[:], in1=l1p[:],
                                        op=ALU.add)
                nc.gpsimd.tensor_tensor(out=OH[:, 2 * C, :], in0=b1[:],
                                        in1=q[:], op=ALU.subtract)
                nc.vector.tensor_copy(OH[:, 2 * C + 1, :], tm[:])
                nc.gpsimd.tensor_tensor(out=OH[:, 2 * C + 2, :],
                                        in0=OH[:, 2 * C, :], in1=tm[:],
                                        op=ALU.mult)

                # ---- PE: packed matmuls accumulate [NRW, NCL] ----
                for i in range(F // PK):
                    fa = i * PK
                    nc.tensor.matmul(
                        acc[:, :],
                        XF[:, fa:fa + PK, :],
                        OH[:, :, fa:fa + PK],
                        start=(k == 0 and i == 0),
                        stop=(k == NCH - 1 and i == F // PK - 1),
                    )

            res = cst.tile([NRW, NCL], F32, tag="res")
            nc.vector.tensor_copy(res[:], acc[:])
            nc.sync.dma_start(acc_d.ap()[:, :], res[:])

    nc.finalize()
    return nc


_CACHE = {}


def _get_program():
    if "nc" not in _CACHE:
        _CACHE["nc"] = build_program()
    return _CACHE["nc"]


def make_in_maps(segin, edgein, segmask, edgemask):
    segin = np.asarray(segin)
    fp8 = mybir.dt.np(FP8)
    in_maps = []
    for k in range(NCORES):
        n, h = k // 2, k % 2
        rs = slice(h * (H // 2), (h + 1) * (H // 2))
        xcm = segin[n, :, rs, :].reshape(C, P, Q)
        xf = np.zeros((P, Q, C + 1), dtype=np.float16)
        xf[:, :, 0:C] = xcm.transpose(1, 2, 0)
        xf[:, :, C] = 1.0
        in_maps.append({
            "xs": xf,
            "xc": np.ascontiguousarray(
                xcm.transpose(1, 0, 2)).astype(fp8),
            "ts": np.ascontiguousarray(
                segmask[n, rs, :].reshape(P, Q)),
            "es": np.ascontiguousarray(
                edgein[n, 0, rs, :].reshape(P, Q)),
            "ms": np.ascontiguousarray(
                edgemask[n, 0, rs, :].reshape(P, Q)),
        })
    return in_maps


def extract_core(acc):
    """acc: [NRW, NCL] f32 psum dump -> dict of per-core partial sums."""
    a = acc.astype(np.float64).reshape(PK, NST, NMV, PK)
    v = np.einsum("fsmf->sm", a)          # [NST, NMV], diag over packed f
    T1 = np.array([v[c, c] for c in range(C)])
    T2 = np.array([v[c, C + c] for c in range(C)])
    B1 = v[C, 0:C]
    B2 = v[C, C:2 * C]
    L1 = v[C + 1, 0:C]
    L2 = v[C + 1, C:2 * C]
    bce_sum = v[C, 2 * C]
    t_sum = v[C, 2 * C + 1]
    bce_t_sum = v[C, 2 * C + 2]
    return {
        "S1": T1 - L1, "S2": T2 - L2, "B1": B1, "B2": B2,
        "bce": bce_sum, "t": t_sum, "bce_t": bce_t_sum,
    }


def combine(acc_list):
    """acc_list: per-core [NRW, NCL] arrays -> final f32 scalar loss."""
    parts = [extract_core(a) for a in acc_list]

    seg_loss = 0.0
    att_loss = 0.0
    for n in range(N):
        pa, pb = parts[2 * n], parts[2 * n + 1]
        S1 = pa["S1"] + pb["S1"]
        S2 = pa["S2"] + pb["S2"]
        bins = pa["B1"] + pb["B1"]
        bins2 = pa["B2"] + pb["B2"]

        w1 = (bins != 0) * (1.0 - bins / HW) + 1.0
        seg_loss += -(w1 * S1).sum() / (w1 * bins).sum()

        vsum = bins2.sum()
        w2 = (bins2 != 0) * (1.0 - bins2 / vsum) + 1.0
        att_loss += -(w2 * S2).sum() / (w2 * bins2).sum()

    pos_bce = sum(p["bce_t"] for p in parts)
    all_bce = sum(p["bce"] for p in parts)
    pos_num = sum(p["t"] for p in parts)
    cnt = float(N * HW)
    neg_num = cnt - pos_num
    neg_bce = all_bce - pos_bce
    ssum = pos_num + neg_num
    edge_loss = (neg_num / ssum * pos_bce + pos_num / ssum * neg_bce) / cnt

    return np.float32(seg_loss + 0.3 * edge_loss + 0.1 * att_loss)


def run_cores(in_maps, trace=False, **kw):
    nc = _get_program()
    res = bass_utils.run_bass_kernel_spmd(
        nc, in_maps, core_ids=list(range(NCORES)), trace=trace, **kw
    )
    return res


def kernel(segin, edgein, segmask, edgemask):
    in_maps = make_in_maps(
        np.asarray(segin), np.asarray(edgein),
        np.asarray(segmask), np.asarray(edgemask))
    res = run_cores(in_maps)
    acc_list = [out["acc"] for out in res.results]
    return combine(acc_list)
